# revision 17
# baseline (speedup 1.0000x reference)
"""DeepBKT 4-layer transformer forward on 8 TRN2 NeuronCores.

Data-parallel over batch: B=16 -> 2 batch items per core; each core runs the
full 4-layer stack on its (2*512, 512) token slab.

v2.1 strategy per core (T = 1024 tokens, D = 512), bf16 matmuls with an
fp32 residual chain (fp32 PSUM accumulation everywhere):
  - residual stream kept in fp32 normal tiles (tokens on partitions) for
    LayerNorm; matmul operands use bf16 copies, transposed (features on
    partitions) by the DMA xbar transpose engine -- no PE transposes.
    Transposes alternate between the SP and ACT HWDGE queues; bulk
    weight/input loads ride the gpsimd SWDGE queue so layer-ahead weight
    prefetch is never stuck behind transposes in a DMA FIFO.
  - causal mask folded into the gram matmul: an extra accumulating matmul
    adds -1e9 to the (key >= query) triangle of each diagonal block, so
    exp() zeroes masked entries with no vector-engine mask multiply.
  - softmax denominators ride as a 65th row of the AV matmul (ones column
    appended to V); reciprocal rows are broadcast across partitions by
    gpsimd.partition_broadcast instead of PE broadcast matmuls.
  - transposed activations use a token-tile-major layout so each DMA
    transpose writes one contiguous [128, 512] block; matmul moving
    operands read them through 3D access patterns.
"""

import numpy as np
import ml_dtypes

import concourse.bass as bass
import concourse.tile as tile
from concourse import bacc, mybir
from concourse.bass_utils import run_bass_kernel_spmd

F32 = mybir.dt.float32
BF = mybir.dt.bfloat16
AF = mybir.ActivationFunctionType
OP = mybir.AluOpType

B, S, D, H, DFF, L = 16, 512, 512, 8, 2048, 4
DK = D // H                       # 64
NCORES = 8
BPC = B // NCORES                 # 2 batch items per core
T = BPC * S                       # 1024 tokens per core
NT = T // 128                     # 8 token tiles
NC = D // 128                     # 4 feature chunks
NF = DFF // 128                   # 16 ffn chunks
EPS = 1e-5
SCALE = 1.0 / np.sqrt(DK)
NEG = -1e9
# (kb, par) -> column offset inside an eT strip [128, 2560]
EO = {(0, 0): 0, (0, 1): 512, (1, 0): 1024, (1, 1): 1408,
      (2, 0): 1792, (2, 1): 2048, (3, 0): 2304, (3, 1): 2432}

_CACHE = {}


def _build(flags):
    nc = bacc.Bacc("TRN2", target_bir_lowering=False, debug=False,
                   num_devices=NCORES)

    d = {}
    d["x0_d"] = nc.dram_tensor("x0", [T, D], BF, kind="ExternalInput")
    d["x0f_d"] = nc.dram_tensor("x0f", [T, D], F32, kind="ExternalInput")
    d["y0_d"] = nc.dram_tensor("y0", [T, D], BF, kind="ExternalInput")
    d["frs_d"] = nc.dram_tensor("frs", [128, T], BF, kind="ExternalInput")
    # cols 0:128 identity, 128:256 lower-tri(-1e9) mask
    d["consts_d"] = nc.dram_tensor("consts", [128, 256], BF, kind="ExternalInput")
    d["wqkv_d"] = nc.dram_tensor("wqkv", [L, 3, 128, NC * D], BF, kind="ExternalInput")
    d["w1_d"] = nc.dram_tensor("w1t", [L, 4, 128, 2048], BF, kind="ExternalInput")
    d["w2_d"] = nc.dram_tensor("w2t", [L, 4, 128, 2048], BF, kind="ExternalInput")
    d["bk_d"] = nc.dram_tensor("bk", [L, D, 1], F32, kind="ExternalInput")
    d["b1_d"] = nc.dram_tensor("b1", [L, DFF, 1], F32, kind="ExternalInput")
    # free-axis vectors pre-broadcast on host to (128, D)
    for nm in ("bvb", "bob", "b2b", "g1b", "be1b", "g2b", "be2b"):
        d[nm + "_d"] = nc.dram_tensor(nm, [L, 128, D], F32, kind="ExternalInput")
    d["out_d"] = nc.dram_tensor("out", [T, D], F32, kind="ExternalOutput")

    with tile.TileContext(nc) as tc:
        _emit(nc, tc, d, flags)
    nc.compile()
    return nc


def _emit(nc, tc, d, flags):
    use_bk, use_bv, use_bo, use_b1, use_b2, use_ln1, use_ln2 = flags
    import contextlib
    ctx = contextlib.ExitStack()
    with ctx:
        sb = ctx.enter_context(tc.tile_pool(name="sb", bufs=1))
        ps = ctx.enter_context(tc.tile_pool(name="ps", bufs=4, space="PSUM"))

        def tl(shape, dtype, tag, bufs, name=None):
            return sb.tile(shape, dtype, tag=tag, bufs=bufs, name=name or tag)

        def pA():
            return ps.tile([128, 512], F32, tag="psA", bufs=4, name="psA")

        def pB():
            return ps.tile([128, 512], F32, tag="psB", bufs=4, name="psB")

        # transposed tensors are token-tile-major: col = tt*512 + c*128 + j,
        # i.e. t[p, tt*512 + c*128 + j] = src[tt*128 + j, c*128 + p]
        def tp_dst(t, tt):  # contiguous [128, 4, 128] destination block
            return t[:, tt * 512:(tt + 1) * 512].rearrange(
                "p (c j) -> p c j", j=128)

        def mv(t, k, b):  # 3D moving operand: tokens b*512.. of chunk k
            return t[:].rearrange("p (tt c j) -> p tt c j", c=NC, j=128)[
                :, b * 4:(b + 1) * 4, k, :]

        def transpose(dst_t, tt, src_ap):
            nc.sync.dma_start(tp_dst(dst_t, tt), src_ap, transpose=True)

        # ---- constants ----
        frs_t = tl([128, T], BF, "frs", 1)
        nc.sync.dma_start(frs_t[:], d["frs_d"].ap())
        consts_t = tl([128, 256], BF, "consts", 1)
        nc.sync.dma_start(consts_t[:], d["consts_d"].ap())
        ident_t = consts_t[:, 0:128]
        masklow_t = consts_t[:, 128:256]
        eps_t = tl([128, 1], F32, "epsb", 1)
        nc.gpsimd.memset(eps_t[:], EPS)

        # ---- load streams: fp32 normal x tiles + transposed bf16 x/y ----
        x_tiles = [tl([128, D], F32, "x", 12) for _ in range(NT)]
        for tt in range(NT):
            nc.gpsimd.dma_start(x_tiles[tt][:], d["x0f_d"].ap()[tt * 128:(tt + 1) * 128, :])
        xT = tl([128, NT * 512], BF, "xT", 2, "xT0")
        yT = tl([128, NT * 512], BF, "yT", 1)
        for tt in range(NT):
            transpose(xT, tt, d["x0_d"].ap()[tt * 128:(tt + 1) * 128, :])
            transpose(yT, tt, d["y0_d"].ap()[tt * 128:(tt + 1) * 128, :])

        def load_qkv(li):
            wk_t = tl([128, NC * D], BF, "wqkv", 4, "wk")
            wv_t = tl([128, NC * D], BF, "wqkv", 4, "wv")
            wo_t = tl([128, NC * D], BF, "wqkv", 4, "wo")
            nc.gpsimd.dma_start(wk_t[:], d["wqkv_d"].ap()[li, 0])
            nc.gpsimd.dma_start(wv_t[:], d["wqkv_d"].ap()[li, 1])
            nc.gpsimd.dma_start(wo_t[:], d["wqkv_d"].ap()[li, 2])
            return wk_t, wv_t, wo_t

        def load_ffn(li):
            w1g = [tl([128, 2048], BF, "w1g", 5) for _ in range(4)]
            w2g = [tl([128, 2048], BF, "w2g", 4) for _ in range(4)]
            for g in range(4):
                nc.gpsimd.dma_start(w1g[g][:], d["w1_d"].ap()[li, g])
                nc.gpsimd.dma_start(w2g[g][:], d["w2_d"].ap()[li, g])
            return w1g, w2g

        qkv_next = load_qkv(0)
        ffn_next = load_ffn(0)
        for li in range(L):
            wk_t, wv_t, wo_t = qkv_next

            bk_t = bv_t = bo_t = b2_t = g1_t = be1_t = g2_t = be2_t = None
            if use_bk:
                bk_t = [tl([128, 1], F32, "bk", 4) for _ in range(NC)]
                for c in range(NC):
                    nc.gpsimd.dma_start(bk_t[c][:], d["bk_d"].ap()[li, c * 128:(c + 1) * 128, :])
            if use_bv:
                bv_t = tl([128, D], F32, "bvb", 1)
                nc.gpsimd.dma_start(bv_t[:], d["bvb_d"].ap()[li])
            if use_bo:
                bo_t = tl([128, D], F32, "bob", 1)
                nc.gpsimd.dma_start(bo_t[:], d["bob_d"].ap()[li])
            if use_b2:
                b2_t = tl([128, D], F32, "b2b", 1)
                nc.gpsimd.dma_start(b2_t[:], d["b2b_d"].ap()[li])
            if use_ln1:
                g1_t = tl([128, D], F32, "g1b", 1)
                be1_t = tl([128, D], F32, "be1b", 1)
                nc.gpsimd.dma_start(g1_t[:], d["g1b_d"].ap()[li])
                nc.gpsimd.dma_start(be1_t[:], d["be1b_d"].ap()[li])
            if use_ln2:
                g2_t = tl([128, D], F32, "g2b", 1)
                be2_t = tl([128, D], F32, "be2b", 1)
                nc.gpsimd.dma_start(g2_t[:], d["g2b_d"].ap()[li])
                nc.gpsimd.dma_start(be2_t[:], d["be2b_d"].ap()[li])

            # ---- V projection -> v_aug tiles (tok, 8*(64 v | 1 one)) ----
            # independent of x: emitted first so the PE has ready work while
            # the previous layer's tail drains.
            v_aug = [tl([128, 8 * 65], BF, "vaug", 8) for _ in range(NT)]
            for tt in range(NT):
                pv = pA()
                for k in range(NC):
                    nc.tensor.matmul(
                        pv[:, 0:512],
                        yT[:, tt * 512 + k * 128:tt * 512 + (k + 1) * 128],
                        wv_t[:, k * 512:(k + 1) * 512],
                        start=(k == 0), stop=(k == NC - 1))
                vdst = v_aug[tt][:].rearrange("p (g e) -> p g e", e=65)[:, :, 0:64]
                vsrc = pv[:, 0:512].rearrange("p (g e) -> p g e", e=64)
                if use_bv:
                    nc.vector.scalar_tensor_tensor(
                        out=vdst, in0=vsrc, scalar=1.0,
                        in1=bv_t[:].rearrange("p (g e) -> p g e", e=64),
                        op0=OP.mult, op1=OP.add)
                else:
                    nc.scalar.copy(vdst, vsrc)
                nc.gpsimd.memset(
                    v_aug[tt][:].rearrange("p (g e) -> p g e", e=65)[:, :, 64:65], 1.0)

            # ---- QK projection (per batch half) ----
            qku = {}
            qks = {}
            for b in range(BPC):
                for c in range(NC):
                    pp = pA()
                    for k in range(NC):
                        nc.tensor.matmul(
                            pp[:, 0:512],
                            wk_t[:, k * 512 + c * 128:k * 512 + (c + 1) * 128],
                            mv(xT, k, b),
                            start=(k == 0), stop=(k == NC - 1))
                    u = tl([128, 512], BF, "qku", 5)
                    s = tl([128, 512], BF, "qks", 5)
                    if use_bk:
                        nc.scalar.activation(u[:], pp[:, 0:512], AF.Identity, bias=bk_t[c][:])
                        nc.vector.scalar_tensor_tensor(
                            out=s[:], in0=pp[:, 0:512], scalar=bk_t[c][:],
                            in1=frs_t[:, b * 512:(b + 1) * 512],
                            op0=OP.add, op1=OP.mult)
                    else:
                        nc.scalar.copy(u[:], pp[:, 0:512])
                        nc.vector.scalar_tensor_tensor(
                            out=s[:], in0=pp[:, 0:512], scalar=1.0,
                            in1=frs_t[:, b * 512:(b + 1) * 512],
                            op0=OP.mult, op1=OP.mult)
                    qku[(c, b)] = u
                    qks[(c, b)] = s

            # ---- attention ----
            uoT = {}
            for b in range(BPC):
                eTs = {}
                # gram + additive causal mask + exp
                for hp in range(NC):
                    eT = eTs[hp] = tl([128, 2560], BF, "eT", 5)
                    for kb in range(4):
                        n = 512 - 128 * kb
                        pg = [pA(), pA()]
                        for par in range(2):
                            r0 = par * 64
                            nc.tensor.matmul(
                                pg[par][:, 0:n],
                                qku[(hp, b)][r0:r0 + 64, 128 * kb:128 * (kb + 1)],
                                qks[(hp, b)][r0:r0 + 64, 128 * kb:512],
                                start=True, stop=False, tile_position=(r0, 0),
                                skip_group_check=True)
                        for par in range(2):
                            # additive -1e9 on the (key >= query) triangle of
                            # the diagonal 128x128 block
                            nc.tensor.matmul(
                                pg[par][:, 0:128], ident_t, masklow_t,
                                start=False, stop=True, skip_group_check=True)
                        for par in range(2):
                            nc.scalar.activation(
                                eT[:, EO[kb, par]:EO[kb, par] + n],
                                pg[par][:, 0:n], AF.Exp)
                # AV + normalization
                for hp in range(NC):
                    eT = eTs[hp]
                    av = [pB(), pB()]
                    for kb in range(4):
                        n = 512 - 128 * kb
                        for par in range(2):
                            h = 2 * hp + par
                            off = EO[kb, par]
                            nc.tensor.matmul(
                                av[par][0:65, 128 * kb:512],
                                v_aug[b * 4 + kb][:, h * 65:(h + 1) * 65],
                                eT[:, off:off + n],
                                start=(kb == 0), stop=(kb == 3),
                                skip_group_check=True)
                    u = uoT[(hp, b)] = tl([128, 512], BF, "uoT", 8)
                    for par in range(2):
                        row1 = tl([1, 512], F32, "row", 4)
                        nc.vector.tensor_scalar_add(row1[:], av[par][64:65, 0:512],
                                                    1e-30)
                        nc.vector.reciprocal_approx_fast(row1[:], row1[:])
                        rb = tl([128, 512], F32, "rb", 4)
                        nc.gpsimd.partition_broadcast(rb[:], row1[:])
                        nc.vector.scalar_tensor_tensor(
                            out=u[par * 64:(par + 1) * 64, :],
                            in0=av[par][0:64, 0:512], scalar=1.0,
                            in1=rb[par * 64:(par + 1) * 64, :],
                            op0=OP.mult, op1=OP.mult)

            if li + 1 < L:
                qkv_next = load_qkv(li + 1)

            # ---- O projection + residual + LN1 (stats batched per half) ----
            x_mid = [None] * NT
            xTm = tl([128, NT * 512], BF, "xTm", 1)
            msum1 = tl([128, NT], F32, "lnst", 8, "msum1")
            sq1 = tl([128, NT], F32, "lnst", 8, "sq1")
            xres1 = [tl([128, D], F32, "xres", 8) for _ in range(NT)]
            for half in range(2):
                for qq in range(4):
                    tt = half * 4 + qq
                    po = pA()
                    for c in range(NC):
                        nc.tensor.matmul(
                            po[:, 0:512],
                            uoT[(c, tt // 4)][:, (tt % 4) * 128:(tt % 4 + 1) * 128],
                            wo_t[:, c * 512:(c + 1) * 512],
                            start=(c == 0), stop=(c == NC - 1))
                    if use_bo:
                        nc.vector.tensor_tensor(po[:, 0:512], po[:, 0:512], bo_t[:], OP.add)
                    nc.vector.scalar_tensor_tensor(
                        out=xres1[tt][:], in0=po[:, 0:512], scalar=1.0,
                        in1=x_tiles[tt][:], op0=OP.mult, op1=OP.add,
                        accum_out=msum1[:, tt:tt + 1])
                    scr = tl([128, D], BF, "scr", 2)
                    nc.scalar.activation(scr[:], xres1[tt][:], AF.Square,
                                         accum_out=sq1[:, tt:tt + 1])
                for pr in range(2):
                    t0 = half * 4 + pr * 2
                    sl = slice(t0, t0 + 2)
                    negmu, rstd = _ln_stats(nc, tl, msum1[:, sl], sq1[:, sl],
                                            eps_t, w=2)
                    for j in range(2):
                        tt = t0 + j
                        xbf = tl([128, D], BF, "xbf", 6)
                        _ln_apply(nc, tl, xres1[tt], xbf,
                                  negmu[:, j:j + 1], rstd[:, j:j + 1],
                                  (g1_t, be1_t) if use_ln1 else None)
                        transpose(xTm, tt, xbf[:])
                    for j in range(2):
                        tt = t0 + j
                        x_mid[tt] = tl([128, D], F32, "x", 12, "xmid")
                        _ln_apply(nc, tl, xres1[tt], x_mid[tt],
                                  negmu[:, j:j + 1], rstd[:, j:j + 1],
                                  (g1_t, be1_t) if use_ln1 else None)

            # ---- FFN (two token-half passes) + residual + LN2 ----
            w1g, w2g = ffn_next
            msum2 = tl([128, NT], F32, "lnst", 8, "msum2")
            sq2 = tl([128, NT], F32, "lnst", 8, "sq2")
            last = li == L - 1
            for half in range(2):
                if half == 1 and not last:
                    ffn_next = load_ffn(li + 1)
                accs = [pB() for _ in range(4)]
                for g in range(4):
                    for fi in range(4):
                        f = g * 4 + fi
                        ph = pA()
                        for k in range(NC):
                            nc.tensor.matmul(
                                ph[:, 0:512],
                                w1g[g][:, fi * 512 + k * 128:fi * 512 + (k + 1) * 128],
                                mv(xTm, k, half),
                                start=(k == 0), stop=(k == NC - 1))
                        hf = tl([128, 512], BF, "hf", 4)
                        if use_b1:
                            b1f = tl([128, 1], F32, "b1f", 3)
                            nc.gpsimd.dma_start(b1f[:], d["b1_d"].ap()[li, f * 128:(f + 1) * 128, :])
                            nc.scalar.activation(hf[:], ph[:, 0:512], AF.Relu, bias=b1f[:])
                        else:
                            nc.scalar.activation(hf[:], ph[:, 0:512], AF.Relu)
                        for q in range(4):
                            nc.tensor.matmul(
                                accs[q][:, 0:512], hf[:, q * 128:(q + 1) * 128],
                                w2g[g][:, fi * 512:(fi + 1) * 512],
                                start=(f == 0), stop=(f == NF - 1))
                xres2 = [tl([128, D], F32, "xres", 8, "xres2") for _ in range(4)]
                for q in range(4):
                    tt = half * 4 + q
                    if use_b2:
                        nc.vector.tensor_tensor(accs[q][:, 0:512], accs[q][:, 0:512],
                                                b2_t[:], OP.add)
                    nc.vector.scalar_tensor_tensor(
                        out=xres2[q][:], in0=accs[q][:, 0:512], scalar=1.0,
                        in1=x_mid[tt][:], op0=OP.mult, op1=OP.add,
                        accum_out=msum2[:, tt:tt + 1])
                    scr = tl([128, D], BF, "scr", 2)
                    nc.scalar.activation(scr[:], xres2[q][:], AF.Square,
                                         accum_out=sq2[:, tt:tt + 1])
                if last:
                    sl = slice(half * 4, half * 4 + 4)
                    negmu, rstd = _ln_stats(nc, tl, msum2[:, sl], sq2[:, sl], eps_t)
                    for q in range(4):
                        tt = half * 4 + q
                        xo = tl([128, D], F32, "x", 12, "xout")
                        _ln_apply(nc, tl, xres2[q], xo,
                                  negmu[:, q:q + 1], rstd[:, q:q + 1],
                                  (g2_t, be2_t) if use_ln2 else None)
                        nc.sync.dma_start(d["out_d"].ap()[tt * 128:(tt + 1) * 128, :],
                                          xo[:])
                else:
                    if half == 0:
                        xT_next = tl([128, NT * 512], BF, "xT", 2, "xTn")
                    for pr in range(2):
                        t0 = half * 4 + pr * 2
                        sl = slice(t0, t0 + 2)
                        negmu, rstd = _ln_stats(nc, tl, msum2[:, sl], sq2[:, sl],
                                                eps_t, w=2)
                        for j in range(2):
                            q = pr * 2 + j
                            tt = t0 + j
                            xbf = tl([128, D], BF, "xbf", 6)
                            _ln_apply(nc, tl, xres2[q], xbf,
                                      negmu[:, j:j + 1], rstd[:, j:j + 1],
                                      (g2_t, be2_t) if use_ln2 else None)
                            transpose(xT_next, tt, xbf[:])
                        for j in range(2):
                            q = pr * 2 + j
                            tt = t0 + j
                            x_tiles[tt] = tl([128, D], F32, "x", 12, "xnew")
                            _ln_apply(nc, tl, xres2[q], x_tiles[tt],
                                      negmu[:, j:j + 1], rstd[:, j:j + 1],
                                      (g2_t, be2_t) if use_ln2 else None)
            if not last:
                xT = xT_next


def _ln_stats(nc, tl, msum, sq, eps_t, w=4):
    """msum/sq: [128, w] fp32 per-tile sums -> (negmu, rstd) [128, w] APs."""
    negmu = tl([128, 4], F32, "lnst", 8, "negmu")[:, 0:w]
    nc.vector.tensor_scalar_mul(negmu, msum, -1.0 / D)
    e2 = tl([128, 4], F32, "lnst", 8, "e2")[:, 0:w]
    nc.vector.tensor_scalar_mul(e2, sq, 1.0 / D)
    mu2 = tl([128, 4], F32, "lnst", 8, "mu2")[:, 0:w]
    nc.vector.tensor_tensor(mu2, negmu, negmu, OP.mult)
    var = tl([128, 4], F32, "lnst", 8, "var")[:, 0:w]
    nc.vector.tensor_tensor(var, e2, mu2, OP.subtract)
    sd = tl([128, 4], F32, "lnst", 8, "sd")[:, 0:w]
    nc.scalar.activation(sd, var, AF.Sqrt, bias=eps_t[:])
    rstd = tl([128, 4], F32, "lnst", 8, "rstd")[:, 0:w]
    nc.vector.reciprocal(rstd, sd)
    return negmu, rstd


def _ln_apply(nc, tl, xres, x_new, negmu, rstd, gb):
    """x_new = (xres + negmu) * rstd [* g + b]."""
    if gb is None:
        nc.vector.tensor_scalar(
            out=x_new[:], in0=xres[:], scalar1=negmu, scalar2=rstd,
            op0=OP.add, op1=OP.mult)
    else:
        g_t, be_t = gb
        xn = tl([128, D], F32, "xn", 2)
        nc.vector.tensor_scalar(
            out=xn[:], in0=xres[:], scalar1=negmu, scalar2=rstd,
            op0=OP.add, op1=OP.mult)
        nc.vector.tensor_tensor(xn[:], xn[:], g_t[:], OP.mult)
        nc.vector.tensor_tensor(x_new[:], xn[:], be_t[:], OP.add)


def _host_prep(inputs):
    bf = ml_dtypes.bfloat16
    q = np.asarray(inputs["q_embed"], np.float32)
    qa = np.asarray(inputs["qa_embed"], np.float32)
    fr = np.asarray(inputs["forget_rate"], np.float32)
    pe = np.asarray(inputs["pe"], np.float32)
    x0f = q + pe
    x0 = x0f.astype(bf)
    y0 = (qa + pe).astype(bf)

    flags = (
        bool(np.any(inputs["bk"])), bool(np.any(inputs["bv"])),
        bool(np.any(inputs["bo"])), bool(np.any(inputs["b1"])),
        bool(np.any(inputs["b2"])),
        bool(np.any(np.asarray(inputs["ln1_g"]) != 1.0) or np.any(inputs["ln1_b"])),
        bool(np.any(np.asarray(inputs["ln2_g"]) != 1.0) or np.any(inputs["ln2_b"])),
    )

    ident = np.eye(128, dtype=np.float32)
    masklow = np.where(np.arange(128)[None, :] <= np.arange(128)[:, None],
                       np.float32(NEG), np.float32(0.0))
    consts = np.concatenate([ident, masklow], axis=1).astype(bf)

    def wtile(w):  # [L, D, D] -> [L, 128, NC*D] with chunk k at cols k*D
        return np.ascontiguousarray(
            np.asarray(w, np.float32).reshape(L, NC, 128, D)
            .transpose(0, 2, 1, 3).reshape(L, 128, NC * D))

    wqkv = np.stack([wtile(inputs["Wk"]), wtile(inputs["Wv"]),
                     wtile(inputs["Wo"])], axis=1).astype(bf)
    # w1t[li, g, p, fi*512 + k*128 + m] = W1[li, 128k+p, 128(4g+fi)+m]
    w1t = (np.asarray(inputs["W1"], np.float32)
           .reshape(L, NC, 128, NF, 128)        # li, k, p, f, m
           .transpose(0, 3, 2, 1, 4)            # li, f, p, k, m
           .reshape(L, 4, 4, 128, 512)          # li, g, fi, p, (k m)
           .transpose(0, 1, 3, 2, 4)            # li, g, p, fi, (k m)
           .reshape(L, 4, 128, 2048)).astype(bf)
    w1t = np.ascontiguousarray(w1t)
    # w2t[li, g, p, fi*512 + m] = W2[li, 128(4g+fi)+p, m]
    w2t = (np.asarray(inputs["W2"], np.float32)
           .reshape(L, 4, 4, 128, 512)          # li, g, fi, p, m
           .transpose(0, 1, 3, 2, 4)            # li, g, p, fi, m
           .reshape(L, 4, 128, 2048)).astype(bf)
    w2t = np.ascontiguousarray(w2t)

    def bcast(v):  # (L, D) -> (L, 128, D)
        v = np.asarray(v, np.float32)
        return np.ascontiguousarray(np.broadcast_to(v[:, None, :], (L, 128, v.shape[-1])))

    common = {
        "wqkv": wqkv, "w1t": w1t, "w2t": w2t, "consts": consts,
        "bk": np.ascontiguousarray(inputs["bk"], np.float32).reshape(L, D, 1),
        "b1": np.ascontiguousarray(inputs["b1"], np.float32).reshape(L, DFF, 1),
        "bvb": bcast(inputs["bv"]), "bob": bcast(inputs["bo"]),
        "b2b": bcast(inputs["b2"]),
        "g1b": bcast(inputs["ln1_g"]), "be1b": bcast(inputs["ln1_b"]),
        "g2b": bcast(inputs["ln2_g"]), "be2b": bcast(inputs["ln2_b"]),
    }

    in_maps = []
    for c in range(NCORES):
        sl = slice(c * BPC, (c + 1) * BPC)
        frs = (fr[sl, :, 0].reshape(1, T) * SCALE).astype(np.float32)
        m = dict(common)
        m["x0"] = np.ascontiguousarray(x0[sl].reshape(T, D))
        m["x0f"] = np.ascontiguousarray(x0f[sl].reshape(T, D).astype(np.float32))
        m["y0"] = np.ascontiguousarray(y0[sl].reshape(T, D))
        m["frs"] = np.ascontiguousarray(np.broadcast_to(frs, (128, T))).astype(bf)
        in_maps.append(m)
    return in_maps, flags


def kernel(_trace=False, **inputs):
    in_maps, flags = _host_prep(inputs)
    if flags not in _CACHE:
        _CACHE[flags] = _build(flags)
    nc = _CACHE[flags]
    br = run_bass_kernel_spmd(nc, in_maps, list(range(NCORES)), trace=_trace)
    out = np.empty((B, S, D), np.float32)
    for c in range(NCORES):
        out[c * BPC:(c + 1) * BPC] = br.results[c]["out"].reshape(BPC, S, D)
    if _trace:
        kernel.last_result = br
    return out
